# revision 1
# baseline (speedup 1.0000x reference)
"""DeepSeekMOE grouped-GEMM kernel for 8 Trainium2 NeuronCores.

Expert-parallel: core g handles expert group g.
Per core:  h = x @ w_up_gate ; act = silu(gate)*up ; out = act @ w_down
with x:[1536,2048], w_up_gate:[2048,2816], w_down:[1408,2048] (fp32).

Dataflow (transpose-free on device):
  - host supplies xT = x.T  ([2048,1536]) so both GEMM operands have the
    contraction dim on partitions.
  - GEMM1 computes hT tiles ([n_chunk 128, m 512]) = w1_colblock.T @ xT,
    so SwiGLU output actT lands directly in [E, M] layout — exactly the
    stationary-operand layout GEMM2 needs. out = actT.T @ w_down comes out
    in natural [M, H] orientation.
All matmuls run in float32r (TF32-class, 1 cycle/row on the PE array).
"""

import sys
import numpy as np

if "/opt/trn_rl_repo" not in sys.path:
    sys.path.insert(0, "/opt/trn_rl_repo")

import concourse.bass as bass
import concourse.bacc as bacc
import concourse.mybir as mybir
import concourse.tile as tile
from concourse.bass_utils import run_bass_kernel_spmd

P = 128
M = 1536          # tokens per expert group
K = 2048          # hidden
N2 = 2816         # 2 * expert_dim (gate | up)
E = 1408          # expert_dim
H = 2048          # hidden (output)

KC = K // P       # 16 contraction chunks, GEMM1
EC = E // P       # 11 contraction chunks, GEMM2 / n-pairs
MT = 512          # m free-dim tile
NMT = M // MT     # 3 m-tiles
MC = M // P       # 12 output m-chunks, GEMM2
HT = 512          # h free-dim tile
NHT = H // HT     # 4 h-tiles

F32 = mybir.dt.float32
F32R = mybir.dt.float32r

_cache = {}


def _build_nc():
    nc = bacc.Bacc("TRN2", target_bir_lowering=False)

    xT = nc.declare_dram_parameter("xT", [K, M], F32R, isOutput=False)
    w1 = nc.declare_dram_parameter("w1", [K, N2], F32R, isOutput=False)
    w2 = nc.declare_dram_parameter("w2", [E, H], F32R, isOutput=False)
    out = nc.declare_dram_parameter("out", [M, H], F32, isOutput=True)

    with tile.TileContext(nc) as tc:
        with tc.tile_pool(name="act", bufs=1) as act_pool, \
             tc.tile_pool(name="ps", bufs=8, space="PSUM") as ps_pool:
            # actT: [E, M] fp32r, resident through both phases (66 KB/part)
            act_t = [act_pool.tile([P, M], F32R, name=f"act{e}", tag=f"act{e}")
                     for e in range(EC)]

            # ---------------- Phase 1: GEMM1 + SwiGLU ----------------
            with tc.tile_pool(name="xt", bufs=1) as xt_pool:
                xts = []
                for k in range(KC):
                    xt = xt_pool.tile([P, M], F32R, name=f"xt{k}", tag=f"xt{k}")
                    nc.sync.dma_start(out=xt, in_=xT[k * P:(k + 1) * P, :])
                    xts.append(xt)

                with tc.tile_pool(name="w1p", bufs=2) as w1_pool, \
                     tc.tile_pool(name="silu", bufs=4) as silu_pool:
                    for i in range(EC):
                        # gate columns i*128, up columns E + i*128
                        wg = w1_pool.tile([P, KC, P], F32R, name=f"wg{i}", tag="wg")
                        wu = w1_pool.tile([P, KC, P], F32R, name=f"wu{i}", tag="wu")
                        nc.sync.dma_start(
                            out=wg,
                            in_=w1[:, i * P:(i + 1) * P]
                                .rearrange("(k p) c -> p k c", p=P))
                        nc.sync.dma_start(
                            out=wu,
                            in_=w1[:, E + i * P:E + (i + 1) * P]
                                .rearrange("(k p) c -> p k c", p=P))

                        ps_g = [ps_pool.tile([P, MT], F32, name=f"psg{i}_{t}", tag="ps")
                                for t in range(NMT)]
                        ps_u = [ps_pool.tile([P, MT], F32, name=f"psu{i}_{t}", tag="ps")
                                for t in range(NMT)]
                        for k in range(KC):
                            for t in range(NMT):
                                nc.tensor.matmul(
                                    ps_g[t], wg[:, k, :],
                                    xts[k][:, t * MT:(t + 1) * MT],
                                    start=(k == 0), stop=(k == KC - 1))
                        for k in range(KC):
                            for t in range(NMT):
                                nc.tensor.matmul(
                                    ps_u[t], wu[:, k, :],
                                    xts[k][:, t * MT:(t + 1) * MT],
                                    start=(k == 0), stop=(k == KC - 1))
                        for t in range(NMT):
                            tmp = silu_pool.tile([P, MT], F32,
                                                 name=f"silu{i}_{t}", tag="silu")
                            nc.scalar.activation(
                                out=tmp, in_=ps_g[t],
                                func=mybir.ActivationFunctionType.Silu)
                            nc.vector.tensor_mul(
                                out=act_t[i][:, t * MT:(t + 1) * MT],
                                in0=tmp, in1=ps_u[t])

            # ---------------- Phase 2: GEMM2 ----------------
            with tc.tile_pool(name="w2p", bufs=1) as w2_pool, \
                 tc.tile_pool(name="ost", bufs=4) as out_pool:
                w2ts = []
                for e in range(EC):
                    w2t = w2_pool.tile([P, H], F32R, name=f"w2t{e}", tag=f"w2t{e}")
                    nc.sync.dma_start(out=w2t, in_=w2[e * P:(e + 1) * P, :])
                    w2ts.append(w2t)

                for mc in range(MC):
                    ps_o = [ps_pool.tile([P, HT], F32, name=f"pso{mc}_{h}", tag="ps")
                            for h in range(NHT)]
                    for e in range(EC):
                        for h in range(NHT):
                            nc.tensor.matmul(
                                ps_o[h],
                                act_t[e][:, mc * P:(mc + 1) * P],
                                w2ts[e][:, h * HT:(h + 1) * HT],
                                start=(e == 0), stop=(e == EC - 1))
                    for h in range(NHT):
                        ot = out_pool.tile([P, HT], F32, name=f"ot{mc}_{h}", tag="ot")
                        nc.vector.tensor_copy(out=ot, in_=ps_o[h])
                        nc.sync.dma_start(
                            out=out[mc * P:(mc + 1) * P, h * HT:(h + 1) * HT],
                            in_=ot)

    nc.compile()
    return nc


def kernel(x, w_up_gate, w_down):
    G = x.shape[0]
    if "nc" not in _cache:
        _cache["nc"] = _build_nc()
    nc = _cache["nc"]

    in_maps = []
    for g in range(G):
        in_maps.append({
            "xT": np.ascontiguousarray(x[g].T),
            "w1": np.ascontiguousarray(w_up_gate[g]),
            "w2": np.ascontiguousarray(w_down[g]),
        })
    res = run_bass_kernel_spmd(nc, in_maps, list(range(G)))
    return np.stack([res.results[g]["out"] for g in range(G)], axis=0)
